# revision 23
# baseline (speedup 1.0000x reference)
"""Multi-head causal attention block on 8 Trainium2 NeuronCores.

Problem: B=4, S=2048, E=1024, H=16, D=64, causal mask, f32.
Sharding: batch (4) x head-group (2 groups of 8 heads) -> 8 cores.
Core c handles batch b=c//2, heads [8g, 8g+8) with g=c%2.
Megatron layout: qkv col-parallel, out_proj row-parallel; the row-parallel
all-reduce (sum of the 2 head-group partial outputs per batch) is done on
host during the gather, as is the output bias.

Per-core dataflow (bf16 matmul operands, f32 PSUM accumulation):
  xT [E,S] (host pre-transposed, bf16) -+-> QT,KT [512,S] (features on parts)
                                        +-> V natural [S,512] + ones column
  scoresT[k,q] = KT_h.T @ QT_h  (2 heads row-tiled in the PE array, K=64
  each -> concurrent on HW via tile_position auto-derivation)
  exp via ACT (scale=1/sqrt(D), bf16 out); no max-subtraction needed
  (|scores| < ~4 for these input scales).
  causal handling at [128k x 512q] block granularity:
    - fully-masked blocks skipped entirely
    - diagonal blocks: score matmul + exp skip the fully-masked column
      prefix; the 128-wide partial wedge is masked multiplicatively on the
      (otherwise idle) Pool engine after exp; the skipped prefix of the
      exp output is zeroed by a Pool memset. The PE stream stays free of
      mask matmuls.
  attn@V: out[65,q] = V_aug.T @ exp_scoresT  (ones col -> row 64 = denom)
  normalize: recip(denom) -> partition-broadcast -> DVE multiply
  out proj: y[s,e] = outT.T @ Wout_rows  (contraction over 512 local feats)
  y stored bf16 (harness sums the two head-group partials on host in f32).

Scheduling: exp keeps ACT ~100% busy during attention, so all other PE
work (QKV projection of the NEXT head pair, V tiles, output projection)
is chopped into ~1us filler thunks with deadlines and emitted one per
attention kt2-group. This removes the multi-us ACT stalls that monolithic
projection blocks caused at pair boundaries. Startup: input DMAs are
interleaved per contraction tile with the first x column-chunk first, so
the first score/exp chain starts after ~4 MB of traffic; a dummy exp at
the top pulls the ACT table load into the DMA phase; Wout loads last.
"""

import numpy as np

B, S, E, H, D = 4, 2048, 1024, 16, 64
HG = H // 2          # heads per group (8)
NP = HG // 2         # head pairs per group (4)
N_CORES = 8
QT_N = 512           # q tile (free dim) in attention
KT_P = 128           # k tile (partitions) in attention
N_QT = S // QT_N     # 4
N_KT = S // KT_P     # 16
F = HG * D           # local features per core (512)

USE_PBC = True       # Pool partition_broadcast instead of PE outer product

_CACHE = {}


def _build(n_et, repeat=1, phases="abc"):
    phases, _, flags = phases.partition("!")
    """Build the Bass module. n_et = number of 128-row contraction tiles of
    the (possibly bias-augmented) embedding dim."""
    import concourse.mybir as mybir
    import concourse.tile as tile
    from concourse import bacc

    dt = mybir.dt
    f32, f32r, bf16 = dt.float32, dt.float32r, dt.bfloat16
    AF = mybir.ActivationFunctionType
    E_pad = n_et * 128
    use_pbc = USE_PBC and "nopbc" not in flags

    nc = bacc.Bacc("TRN2", target_bir_lowering=False, debug=False,
                   enable_asserts=False, num_devices=N_CORES)

    XT = nc.dram_tensor("xt", [E_pad, S], bf16, kind="ExternalInput").ap()
    WQKV = nc.dram_tensor("wqkv", [E_pad, 3 * F], bf16, kind="ExternalInput").ap()
    WOUT = nc.dram_tensor("wout", [F, E], bf16, kind="ExternalInput").ap()
    DMASK = nc.dram_tensor("dmask", [128, 4, 128], bf16, kind="ExternalInput").ap()
    ONES = nc.dram_tensor("ones64", [1, 64], f32r, kind="ExternalInput").ap()
    Y = nc.dram_tensor("y", [S, E], bf16, kind="ExternalOutput").ap()

    with tile.TileContext(nc) as tc, \
         nc.allow_low_precision(reason="bf16 matmul operands by design"):
      for _rep in range(repeat):
        with tc.tile_pool(name="persist", bufs=1) as persist, \
             tc.tile_pool(name="mm_ps", bufs=2, space="PSUM") as mm_ps, \
             tc.tile_pool(name="sp_ps", bufs=2, space="PSUM") as sp_ps, \
             tc.tile_pool(name="at_ps", bufs=1, space="PSUM") as at_ps, \
             tc.tile_pool(name="exp_sb", bufs=6) as exp_sb, \
             tc.tile_pool(name="nrm_sb", bufs=4) as nrm_sb, \
             tc.tile_pool(name="y_sb", bufs=4) as y_sbp:
            # persistent SBUF tensors (all bf16 except the f32r ones row)
            xk = [persist.tile([128, S], bf16, tag=f"xk{e}", name=f"xk{e}")
                  for e in range(n_et)]
            wqk = [persist.tile([128, 2 * F], bf16, tag=f"wqk{e}", name=f"wqk{e}")
                   for e in range(n_et)]
            wv = [persist.tile([128, F], bf16, tag=f"wv{e}", name=f"wv{e}")
                  for e in range(n_et)]
            wout_sb = [persist.tile([128, E], bf16, tag=f"wo{p}", name=f"wo{p}")
                       for p in range(NP)]
            qt_sb = [persist.tile([128, S], bf16, tag=f"qt{p}", name=f"qt{p}")
                     for p in range(NP)]
            kt_sb = [persist.tile([128, S], bf16, tag=f"kt{p}", name=f"kt{p}")
                     for p in range(NP)]
            vav = [persist.tile([128, HG * (D + 1)], bf16, tag=f"va{k}",
                                name=f"va{k}") for k in range(N_KT)]
            outt = [persist.tile([128, S], bf16, tag=f"ot{p}", name=f"ot{p}")
                    for p in range(NP)]
            masks = persist.tile([128, 4, 128], bf16, tag="masks")
            ones_sb = persist.tile([1, 64], f32r, tag="ones")
            scratch = persist.tile([1, 8], f32, tag="scratch")

            # dummy exp: pulls the ~2.7us ACT table load into the DMA phase
            nc.gpsimd.memset(scratch[:], 0.0)
            nc.scalar.activation(scratch[:], scratch[:], AF.Exp)

            # input DMAs. Issue cost (~0.6us per dma_start on one sequencer)
            # gates startup, so issue is spread over three DMA-capable
            # queues (SP, ACT, gpsimd/SWDGE) and the first batch carries
            # only what attention(pair0, qt0) needs. Pair-0's two Q/K
            # weight-column blocks (cols [0:128] and [F:F+128], stride F)
            # are fetched as ONE strided DMA each.
            def wqk_pair_ap(t, p):
                # pair p's Q and K weight columns ([128p:128p+128] and
                # [F+128p:F+128p+128]) as one strided 2-block pattern
                return t[:, 128 * p:F + 128 * (p + 1)].rearrange(
                    "p (b c) -> p b c", c=128)[:, 0::NP, :]

            nc.scalar.dma_start(masks[:], DMASK[:])
            nc.scalar.dma_start(ones_sb[:], ONES[:])
            for e in range(n_et):
                nc.sync.dma_start(wqk_pair_ap(wqk[e], 0),
                                  wqk_pair_ap(WQKV[128 * e:128 * (e + 1), :], 0))
                nc.scalar.dma_start(xk[e][:, 0:QT_N],
                                    XT[128 * e:128 * (e + 1), 0:QT_N])
            for e in range(n_et):
                nc.gpsimd.dma_start(wv[e][:],
                                    WQKV[128 * e:128 * (e + 1), 2 * F:3 * F])
            for p in range(1, NP):
                for e in range(n_et):
                    nc.sync.dma_start(
                        wqk_pair_ap(wqk[e], p),
                        wqk_pair_ap(WQKV[128 * e:128 * (e + 1), :], p))
            for e in range(n_et):
                nc.gpsimd.dma_start(xk[e][:, QT_N:S],
                                    XT[128 * e:128 * (e + 1), QT_N:S])
            for p in range(NP):
                nc.sync.dma_start(wout_sb[p][:], WOUT[128 * p:128 * (p + 1), :])

            def emit_v(st):
                """V tile st (natural layout + ones column)."""
                ps = mm_ps.tile([128, 512], f32, tag="mm", name="mmps")
                for e in range(n_et):
                    nc.tensor.matmul(
                        ps[:],
                        xk[e][:, 128 * st:128 * (st + 1)],
                        wv[e][:],
                        start=(e == 0), stop=(e == n_et - 1))
                va3 = vav[st].rearrange("p (h c) -> p h c", c=D + 1)
                nc.vector.tensor_copy(
                    va3[:, :, 0:D],
                    ps[:].rearrange("p (h c) -> p h c", c=D))
                nc.gpsimd.memset(va3[:, :, D:D + 1], 1.0)

            def emit_qk_half(p, which, sc):
                """One 512-column chunk of the QT (which=0) or KT (which=1)
                projection for head pair p."""
                dest = qt_sb[p] if which == 0 else kt_sb[p]
                ft = p if which == 0 else NP + p
                ps = mm_ps.tile([128, 512], f32, tag="mm", name="mmps")
                for e in range(n_et):
                    nc.tensor.matmul(
                        ps[:],
                        wqk[e][:, 128 * ft:128 * (ft + 1)],
                        xk[e][:, 512 * sc:512 * (sc + 1)],
                        start=(e == 0), stop=(e == n_et - 1))
                nc.vector.tensor_copy(dest[:, 512 * sc:512 * (sc + 1)], ps[:])

            def emit_proj_tile(st, et):
                """One [128s x 512e] output-projection tile."""
                ps = mm_ps.tile([128, 512], f32, tag="mm", name="mmps")
                for p in range(NP):
                    nc.tensor.matmul(
                        ps[:],
                        outt[p][:, 128 * st:128 * (st + 1)],
                        wout_sb[p][:, 512 * et:512 * (et + 1)],
                        start=(p == 0), stop=(p == NP - 1))
                ysb = y_sbp.tile([128, 512], bf16, tag="ysb")
                nc.vector.tensor_copy(ysb[:], ps[:])
                nc.sync.dma_start(
                    Y[128 * st:128 * (st + 1), 512 * et:512 * (et + 1)],
                    ysb[:])

            # filler queue: (deadline_key, thunk); flush(k) emits everything
            # with deadline <= k, pop_one() emits the nearest-deadline thunk.
            fillers = []

            def push(key, thunk):
                fillers.append((key, thunk))
                fillers.sort(key=lambda kt_: kt_[0])

            def flush(key):
                while fillers and fillers[0][0] <= key:
                    fillers.pop(0)[1]()

            def pop_one():
                if fillers:
                    fillers.pop(0)[1]()

            def emit_attn(p, qt):
                """Attention for head pair p, queries [512*qt, 512*(qt+1)).

                Two k-tiles of one head share a 2-bank scores PSUM tile
                (each matmul stays inside its own bank) so a single exp
                instruction covers ~1024 columns: half the ACT instruction
                overhead per unit of work. One filler thunk is emitted per
                kt2 group to keep PE fed while ACT runs exp."""
                kt_max = (qt + 1) * (QT_N // KT_P)
                apA = at_ps.tile([128, QT_N], f32, tag="apA")
                apB = at_ps.tile([128, QT_N], f32, tag="apB")
                nomask = "nomask" in flags
                for kt2 in range(kt_max // 2):
                    sps, eps2, lo_e = {}, {}, 0
                    for hh in range(2):
                        lo, hi = 64 * hh, 64 * hh + 64
                        sp = sp_ps.tile([128, 2 * QT_N], f32, tag="sp2",
                                        name="sp2")
                        sps[hh] = sp
                        for sub in range(2):
                            kt = 2 * kt2 + sub
                            dlt = kt - (qt * QT_N) // KT_P
                            # skip the fully-masked 128*dlt column prefix
                            # of diagonal blocks
                            c0 = 128 * dlt if (dlt > 0 and not nomask) else 0
                            if sub == 0:
                                lo_e = c0
                            nc.tensor.matmul(
                                sp[:, QT_N * sub + c0:QT_N * (sub + 1)],
                                kt_sb[p][lo:hi, 128 * kt:128 * (kt + 1)],
                                qt_sb[p][lo:hi,
                                         QT_N * qt + c0:QT_N * (qt + 1)],
                                start=True, stop=True)
                    for hh in range(2):
                        ep = exp_sb.tile([128, 2 * QT_N], bf16, tag=f"e{hh}",
                                         name=f"e{hh}")
                        eps2[hh] = ep
                        nc.scalar.activation(
                            ep[:, lo_e:2 * QT_N], sps[hh][:, lo_e:2 * QT_N],
                            AF.Copy if "noexp" in flags else AF.Exp,
                            scale=float(1.0 / np.sqrt(D)))
                        for sub in range(2):
                            kt = 2 * kt2 + sub
                            dlt = kt - (qt * QT_N) // KT_P
                            if dlt >= 0 and not nomask:
                                w0 = QT_N * sub + 128 * dlt
                                if dlt > 0:
                                    nc.gpsimd.memset(
                                        ep[:, QT_N * sub:w0], 0.0)
                                # wedge: multiplicative 0/1 causal mask
                                nc.gpsimd.tensor_mul(
                                    ep[:, w0:w0 + 128],
                                    ep[:, w0:w0 + 128],
                                    masks[:, dlt, :])
                    for hh, ap in ((0, apA), (1, apB)):
                        h = 2 * p + hh
                        for sub in range(2):
                            kt = 2 * kt2 + sub
                            dlt = kt - (qt * QT_N) // KT_P
                            # ep columns below the diagonal block are zero;
                            # kt=0 (never trimmed) start-writes the full
                            # accumulator range, later kt accumulate into
                            # the valid suffix only
                            cv = 128 * dlt if (dlt > 0 and not nomask) else 0
                            nc.tensor.matmul(
                                ap[0:D + 1, cv:QT_N],
                                vav[kt][:, (D + 1) * h:(D + 1) * (h + 1)],
                                eps2[hh][:, QT_N * sub + cv:QT_N * (sub + 1)],
                                start=(kt == 0), stop=(kt == kt_max - 1))
                    pop_one()
                    if qt == 0:
                        pop_one()
                for hh, ap in ((0, apA), (1, apB)):
                    rec = nrm_sb.tile([1, QT_N], f32r, tag="rec")
                    nc.vector.reciprocal(rec[:], ap[D:D + 1, :])
                    bsb = nrm_sb.tile([64, QT_N], f32r, tag="bsb")
                    if use_pbc:
                        nc.gpsimd.partition_broadcast(bsb[:], rec[:])
                    else:
                        bps = mm_ps.tile([64, QT_N], f32, tag="mm", name="mmps")
                        nc.tensor.matmul(bps[:], ones_sb[:], rec[:],
                                         start=True, stop=True)
                        nc.vector.tensor_copy(bsb[:], bps[:])
                    nc.vector.tensor_mul(
                        outt[p][64 * hh:64 * hh + 64, QT_N * qt:QT_N * (qt + 1)],
                        ap[0:D, :], bsb[:])

            if phases == "a":
                for st in range(N_KT):
                    emit_v(st)
                for p in range(NP):
                    for sc in range(N_QT):
                        emit_qk_half(p, 0, sc)
                        emit_qk_half(p, 1, sc)
                for p in range(NP):
                    nc.sync.dma_start(
                        Y[(2 * p) * 128:(2 * p + 1) * 128, :],
                        qt_sb[p][:, 0:E])
                    nc.sync.dma_start(
                        Y[(2 * p + 1) * 128:(2 * p + 2) * 128, :],
                        kt_sb[p][:, 0:E])
                continue

            # qt-MAJOR schedule: all 4 head pairs at q-range qt, then the
            # output projection of qt. proj work spreads evenly across the
            # whole attention phase (the qt3 row alone has 32 kt2 groups to
            # hide proj(2) in) and the post-last-exp tail is just proj(3).
            # deadline key = sequential attention-slot index 4*qt + p.
            for p in range(NP):
                for sc in range(N_QT):
                    key = 4 * sc + p
                    push(key, lambda p=p, sc=sc: emit_qk_half(p, 1, sc))
                    push(key, lambda p=p, sc=sc: emit_qk_half(p, 0, sc))
            for st in range(N_KT):
                push(4 * (st // 4), lambda st=st: emit_v(st))
            for qt in range(N_QT):
                for p in range(NP):
                    flush(4 * qt + p)
                    emit_attn(p, qt)
                if phases == "abc":
                    for st in range(4 * qt, 4 * (qt + 1)):
                        for et in range(E // 512):
                            push(99, lambda st=st, et=et:
                                 emit_proj_tile(st, et))
            flush(99)

            if phases == "ab":
                for p in range(NP):
                    nc.sync.dma_start(
                        Y[(2 * p) * 128:(2 * p + 1) * 128, :],
                        outt[p][:, 0:E])

    nc.compile()
    return nc


def _get_nc(n_et, repeat=1, phases="abc"):
    key = (n_et, repeat, phases)
    if key not in _CACHE:
        _CACHE[key] = _build(n_et, repeat, phases)
    return _CACHE[key]


def _shard(x, mask, Wqkv, bqkv, Wout, bout):
    """Host-side sharding: per-core input dicts."""
    import ml_dtypes

    bf16 = ml_dtypes.bfloat16
    x = np.asarray(x, dtype=np.float32)
    mask = np.asarray(mask)
    Wqkv = np.asarray(Wqkv, dtype=np.float32)
    bqkv = np.asarray(bqkv, dtype=np.float32)
    Wout = np.asarray(Wout, dtype=np.float32)

    has_bias = bool(np.any(bqkv))
    n_et = 9 if has_bias else 8
    E_pad = n_et * 128

    # diagonal wedge mask tiles (multiplicative 0/1), 128 columns each:
    # dmask[i, d, j] corresponds to mask[128*d + j, 128*d + i] -- the
    # partially-masked 128-wide wedge of diagonal block d within a q-tile.
    dmask = np.stack(
        [np.asarray(mask[128 * d:128 * (d + 1), 128 * d:128 * (d + 1)].T)
         for d in range(4)], axis=1).astype(np.float32)
    dmask = np.ascontiguousarray(dmask.astype(bf16))  # [128, 4, 128]

    in_maps = []
    for c in range(N_CORES):
        b, g = divmod(c, 2)
        heads = range(HG * g, HG * (g + 1))
        # per-group weight slices, feature order [Q heads | K heads | V heads]
        cols = []
        for blk in range(3):  # q, k, v blocks of Wqkv
            for h in heads:
                cols.append(Wqkv[:, blk * E + D * h: blk * E + D * h + D])
        wqkv_c = np.concatenate(cols, axis=1)  # [E, 3F]
        if has_bias:
            bias_cols = []
            for blk in range(3):
                for h in heads:
                    bias_cols.append(bqkv[blk * E + D * h: blk * E + D * h + D])
            brow = np.concatenate(bias_cols)[None, :]  # [1, 3F]
            wqkv_c = np.concatenate(
                [wqkv_c, brow, np.zeros((E_pad - E - 1, 3 * F), np.float32)], axis=0)
        xt_c = np.ascontiguousarray(x[b].T)  # [E, S]
        if has_bias:
            aug = np.zeros((E_pad - E, S), np.float32)
            aug[0, :] = 1.0
            xt_c = np.concatenate([xt_c, aug], axis=0)
        wout_c = np.ascontiguousarray(Wout[F * g:F * (g + 1), :])  # [F, E]
        in_maps.append({
            "xt": np.ascontiguousarray(xt_c.astype(bf16)),
            "wqkv": np.ascontiguousarray(wqkv_c.astype(bf16)),
            "wout": np.ascontiguousarray(wout_c.astype(bf16)),
            "dmask": dmask,
            "ones64": np.ones((1, 64), np.float32),
        })
    return in_maps, n_et


def run_sharded(inputs, trace=False):
    """Run the SPMD kernel; returns (y_full [B,S,E] f32, BassKernelResults)."""
    from concourse.bass_utils import run_bass_kernel_spmd

    in_maps, n_et = _shard(**inputs)
    nc = _get_nc(n_et)
    res = run_bass_kernel_spmd(nc, in_maps, core_ids=list(range(N_CORES)),
                               trace=trace)
    bout = np.asarray(inputs["bout"], dtype=np.float32)
    y = np.empty((B, S, E), np.float32)
    for b in range(B):
        y[b] = (res.results[2 * b]["y"].astype(np.float32)
                + res.results[2 * b + 1]["y"].astype(np.float32) + bout)
    return y, res


def kernel(**inputs) -> np.ndarray:
    y, _ = run_sharded(inputs, trace=False)
    return y


# revision 31
# speedup vs baseline: 1.1159x; 1.1159x over previous
"""Multi-head causal attention block on 8 Trainium2 NeuronCores.

Problem: B=4, S=2048, E=1024, H=16, D=64, causal mask, f32.
Sharding: batch (4) x head-group (2 groups of 8 heads) -> 8 cores.
Core c handles batch b=c//2, heads [8g, 8g+8) with g=c%2.
Megatron layout: qkv col-parallel, out_proj row-parallel; the row-parallel
all-reduce (sum of the 2 head-group partial outputs per batch) is done on
host during the gather, as is the output bias.

Per-core dataflow (bf16 matmul operands, f32 PSUM accumulation):
  xT [E,S] (host pre-transposed, bf16) -+-> QT,KT [512,S] (features on parts)
                                        +-> V natural [S,512] + ones column
  scoresT[k,q] = KT_h.T @ QT_h  (2 heads row-tiled in the PE array, K=64
  each -> concurrent on HW via tile_position auto-derivation)
  exp via ACT (scale=1/sqrt(D), bf16 out); no max-subtraction needed
  (|scores| < ~4 for these input scales).
  causal handling at [128k x 512q] block granularity:
    - fully-masked blocks skipped entirely
    - diagonal blocks: score matmul + exp skip the fully-masked column
      prefix; the 128-wide partial wedge is masked multiplicatively on the
      (otherwise idle) Pool engine after exp; the skipped prefix of the
      exp output is zeroed by a Pool memset. The PE stream stays free of
      mask matmuls.
  attn@V: out[65,q] = V_aug.T @ exp_scoresT  (ones col -> row 64 = denom)
  normalize: recip(denom) -> partition-broadcast -> DVE multiply
  out proj: y[s,e] = outT.T @ Wout_rows  (contraction over 512 local feats)
  y stored bf16 (harness sums the two head-group partials on host in f32).

Scheduling: exp keeps ACT ~100% busy during attention, so all other PE
work (QKV projection of the NEXT head pair, V tiles, output projection)
is chopped into ~1us filler thunks with deadlines and emitted one per
attention kt2-group. This removes the multi-us ACT stalls that monolithic
projection blocks caused at pair boundaries. Startup: input DMAs are
interleaved per contraction tile with the first x column-chunk first, so
the first score/exp chain starts after ~4 MB of traffic; a dummy exp at
the top pulls the ACT table load into the DMA phase; Wout loads last.
"""

import numpy as np

B, S, E, H, D = 4, 2048, 1024, 16, 64
HG = H // 2          # heads per group (8)
NP = HG // 2         # head pairs per group (4)
N_CORES = 8
QT_N = 512           # q tile (free dim) in attention
KT_P = 128           # k tile (partitions) in attention
N_QT = S // QT_N     # 4
N_KT = S // KT_P     # 16
F = HG * D           # local features per core (512)

USE_PBC = True       # Pool partition_broadcast instead of PE outer product

_CACHE = {}


def _build(n_et, repeat=1, phases="abc"):
    phases, _, flags = phases.partition("!")
    """Build the Bass module. n_et = number of 128-row contraction tiles of
    the (possibly bias-augmented) embedding dim."""
    import concourse.mybir as mybir
    import concourse.tile as tile
    from concourse import bacc

    dt = mybir.dt
    f32, f32r, bf16 = dt.float32, dt.float32r, dt.bfloat16
    AF = mybir.ActivationFunctionType
    E_pad = n_et * 128
    use_pbc = USE_PBC and "nopbc" not in flags

    nc = bacc.Bacc("TRN2", target_bir_lowering=False, debug=False,
                   enable_asserts=False, num_devices=N_CORES)

    XT = nc.dram_tensor("xt", [E_pad, S], bf16, kind="ExternalInput").ap()
    WQKV = nc.dram_tensor("wqkv", [E_pad, 3 * F], bf16, kind="ExternalInput").ap()
    WOUT = nc.dram_tensor("wout", [F, E], bf16, kind="ExternalInput").ap()
    DMASK = nc.dram_tensor("dmask", [128, 4, 128], bf16, kind="ExternalInput").ap()
    ONES = nc.dram_tensor("ones64", [1, 64], f32r, kind="ExternalInput").ap()
    Y = nc.dram_tensor("y", [S, E], bf16, kind="ExternalOutput").ap()

    with tile.TileContext(nc) as tc, \
         nc.allow_low_precision(reason="bf16 matmul operands by design"):
      for _rep in range(repeat):
        with tc.tile_pool(name="persist", bufs=1) as persist, \
             tc.tile_pool(name="mm_ps", bufs=2, space="PSUM") as mm_ps, \
             tc.tile_pool(name="sp_ps", bufs=2, space="PSUM") as sp_ps, \
             tc.tile_pool(name="at_ps", bufs=1, space="PSUM") as at_ps, \
             tc.tile_pool(name="exp_sb", bufs=6) as exp_sb, \
             tc.tile_pool(name="nrm_sb", bufs=4) as nrm_sb, \
             tc.tile_pool(name="y_sb", bufs=4) as y_sbp:
            # persistent SBUF tensors (all bf16 except the f32r ones row)
            xk = [persist.tile([128, S], bf16, tag=f"xk{e}", name=f"xk{e}")
                  for e in range(n_et)]
            wqk = [persist.tile([128, 2 * F], bf16, tag=f"wqk{e}", name=f"wqk{e}")
                   for e in range(n_et)]
            wv = [persist.tile([128, F], bf16, tag=f"wv{e}", name=f"wv{e}")
                  for e in range(n_et)]
            wout_sb = [persist.tile([128, E], bf16, tag=f"wo{p}", name=f"wo{p}")
                       for p in range(NP)]
            qt_sb = [persist.tile([128, S], bf16, tag=f"qt{p}", name=f"qt{p}")
                     for p in range(NP)]
            kt_sb = [persist.tile([128, S], bf16, tag=f"kt{p}", name=f"kt{p}")
                     for p in range(NP)]
            vav = [persist.tile([128, HG * (D + 1)], bf16, tag=f"va{k}",
                                name=f"va{k}") for k in range(N_KT)]
            outt = [persist.tile([128, S], bf16, tag=f"ot{p}", name=f"ot{p}")
                    for p in range(NP)]
            masks = persist.tile([128, 4, 128], bf16, tag="masks")
            ones_sb = persist.tile([1, 64], f32r, tag="ones")
            scratch = persist.tile([1, 8], f32, tag="scratch")

            # dummy exp: pulls the ~2.7us ACT table load into the DMA phase
            nc.gpsimd.memset(scratch[:], 0.0)
            nc.scalar.activation(scratch[:], scratch[:], AF.Exp)

            # input DMAs. Issue cost (~0.6us per dma_start on one sequencer)
            # gates startup, so issue is spread over three DMA-capable
            # queues (SP, ACT, gpsimd/SWDGE) and the first batch carries
            # only what attention(pair0, qt0) needs. Pair-0's two Q/K
            # weight-column blocks (cols [0:128] and [F:F+128], stride F)
            # are fetched as ONE strided DMA each.
            def wqk_pair_ap(t, p):
                # pair p's Q and K weight columns ([128p:128p+128] and
                # [F+128p:F+128p+128]) as one strided 2-block pattern
                return t[:, 128 * p:F + 128 * (p + 1)].rearrange(
                    "p (b c) -> p b c", c=128)[:, 0::NP, :]

            nc.scalar.dma_start(masks[:], DMASK[:])
            nc.scalar.dma_start(ones_sb[:], ONES[:])
            # the first exp chain needs only pair-0 Q/K weights + the first
            # x column chunk (1.5 MB): split that set across both HWDGE
            # queues ahead of everything else so it isn't bandwidth-starved
            # by the bulk transfers.
            for e in range(n_et):
                q0 = nc.sync if e % 2 == 0 else nc.scalar
                q0.dma_start(wqk_pair_ap(wqk[e], 0),
                             wqk_pair_ap(WQKV[128 * e:128 * (e + 1), :], 0))
                q0.dma_start(xk[e][:, 0:QT_N],
                             XT[128 * e:128 * (e + 1), 0:QT_N])
            for e in range(n_et):
                nc.gpsimd.dma_start(wv[e][:],
                                    WQKV[128 * e:128 * (e + 1), 2 * F:3 * F])
            for p in range(1, NP):
                for e in range(n_et):
                    nc.sync.dma_start(
                        wqk_pair_ap(wqk[e], p),
                        wqk_pair_ap(WQKV[128 * e:128 * (e + 1), :], p))
            for e in range(n_et):
                nc.gpsimd.dma_start(xk[e][:, QT_N:S],
                                    XT[128 * e:128 * (e + 1), QT_N:S])

            def emit_v(st):
                """V tile st (natural layout + ones column)."""
                ps = mm_ps.tile([128, 512], f32, tag="mm", name="mmps")
                for e in range(n_et):
                    nc.tensor.matmul(
                        ps[:],
                        xk[e][:, 128 * st:128 * (st + 1)],
                        wv[e][:],
                        start=(e == 0), stop=(e == n_et - 1))
                va3 = vav[st].rearrange("p (h c) -> p h c", c=D + 1)
                nc.vector.tensor_copy(
                    va3[:, :, 0:D],
                    ps[:].rearrange("p (h c) -> p h c", c=D))
                nc.gpsimd.memset(va3[:, :, D:D + 1], 1.0)

            def emit_qk_half(p, which, sc):
                """One 512-column chunk of the QT (which=0) or KT (which=1)
                projection for head pair p."""
                dest = qt_sb[p] if which == 0 else kt_sb[p]
                ft = p if which == 0 else NP + p
                ps = mm_ps.tile([128, 512], f32, tag="mm", name="mmps")
                for e in range(n_et):
                    nc.tensor.matmul(
                        ps[:],
                        wqk[e][:, 128 * ft:128 * (ft + 1)],
                        xk[e][:, 512 * sc:512 * (sc + 1)],
                        start=(e == 0), stop=(e == n_et - 1))
                nc.vector.tensor_copy(dest[:, 512 * sc:512 * (sc + 1)], ps[:])

            def emit_proj_tile(st, et):
                """One [128s x 512e] output-projection tile."""
                ps = mm_ps.tile([128, 512], f32, tag="mm", name="mmps")
                for p in range(NP):
                    nc.tensor.matmul(
                        ps[:],
                        outt[p][:, 128 * st:128 * (st + 1)],
                        wout_sb[p][:, 512 * et:512 * (et + 1)],
                        start=(p == 0), stop=(p == NP - 1))
                ysb = y_sbp.tile([128, 512], bf16, tag="ysb")
                nc.vector.tensor_copy(ysb[:], ps[:])
                nc.sync.dma_start(
                    Y[128 * st:128 * (st + 1), 512 * et:512 * (et + 1)],
                    ysb[:])

            # filler queue: (deadline_key, thunk); flush(k) emits everything
            # with deadline <= k, pop_one() emits the nearest-deadline thunk.
            fillers = []

            def push(key, thunk):
                fillers.append((key, thunk))
                fillers.sort(key=lambda kt_: kt_[0])

            def flush(key):
                while fillers and fillers[0][0] <= key:
                    fillers.pop(0)[1]()

            def pop_one(limit=None):
                if fillers and (limit is None or fillers[0][0] <= limit):
                    fillers.pop(0)[1]()

            def emit_attn(p, qt):
                """Attention for head pair p, queries [512*qt, 512*(qt+1)).

                Two k-tiles of one head share a 2-bank scores PSUM tile
                (each matmul stays inside its own bank) so a single exp
                instruction covers ~1024 columns: half the ACT instruction
                overhead per unit of work. One filler thunk is emitted per
                kt2 group to keep PE fed while ACT runs exp."""
                kt_max = (qt + 1) * (QT_N // KT_P)
                apA = at_ps.tile([128, QT_N], f32, tag="apA")
                apB = at_ps.tile([128, QT_N], f32, tag="apB")
                nomask = "nomask" in flags
                for kt2 in range(kt_max // 2):
                    sps, eps2, lo_e = {}, {}, 0
                    for hh in range(2):
                        lo, hi = 64 * hh, 64 * hh + 64
                        sp = sp_ps.tile([128, 2 * QT_N], f32, tag="sp2",
                                        name="sp2")
                        sps[hh] = sp
                        for sub in range(2):
                            kt = 2 * kt2 + sub
                            dlt = kt - (qt * QT_N) // KT_P
                            # skip the fully-masked 128*dlt column prefix
                            # of diagonal blocks
                            c0 = 128 * dlt if (dlt > 0 and not nomask) else 0
                            if sub == 0:
                                lo_e = c0
                            nc.tensor.matmul(
                                sp[:, QT_N * sub + c0:QT_N * (sub + 1)],
                                kt_sb[p][lo:hi, 128 * kt:128 * (kt + 1)],
                                qt_sb[p][lo:hi,
                                         QT_N * qt + c0:QT_N * (qt + 1)],
                                start=True, stop=True)
                    for hh in range(2):
                        ep = exp_sb.tile([128, 2 * QT_N], bf16, tag=f"e{hh}",
                                         name=f"e{hh}")
                        eps2[hh] = ep
                        nc.scalar.activation(
                            ep[:, lo_e:2 * QT_N], sps[hh][:, lo_e:2 * QT_N],
                            AF.Copy if "noexp" in flags else AF.Exp,
                            scale=float(1.0 / np.sqrt(D)))
                        for sub in range(2):
                            kt = 2 * kt2 + sub
                            dlt = kt - (qt * QT_N) // KT_P
                            if dlt >= 0 and not nomask:
                                w0 = QT_N * sub + 128 * dlt
                                if dlt > 0:
                                    nc.gpsimd.memset(
                                        ep[:, QT_N * sub:w0], 0.0)
                                # wedge: multiplicative 0/1 causal mask
                                nc.gpsimd.tensor_mul(
                                    ep[:, w0:w0 + 128],
                                    ep[:, w0:w0 + 128],
                                    masks[:, dlt, :])
                    for hh, ap in ((0, apA), (1, apB)):
                        h = 2 * p + hh
                        for sub in range(2):
                            kt = 2 * kt2 + sub
                            dlt = kt - (qt * QT_N) // KT_P
                            # ep columns below the diagonal block are zero;
                            # kt=0 (never trimmed) start-writes the full
                            # accumulator range, later kt accumulate into
                            # the valid suffix only
                            cv = 128 * dlt if (dlt > 0 and not nomask) else 0
                            nc.tensor.matmul(
                                ap[0:D + 1, cv:QT_N],
                                vav[kt][:, (D + 1) * h:(D + 1) * (h + 1)],
                                eps2[hh][:, QT_N * sub + cv:QT_N * (sub + 1)],
                                start=(kt == 0), stop=(kt == kt_max - 1))
                    # bounded-horizon pops: near-deadline work only, so
                    # proj tiles (key 16) wait for the ACT-bound qt3 row
                    pop_one(4 * qt + p + 4)
                    if qt == 0:
                        pop_one(4 * qt + p + 4)
                for hh, ap in ((0, apA), (1, apB)):
                    rec = nrm_sb.tile([1, QT_N], f32r, tag="rec")
                    nc.vector.reciprocal(rec[:], ap[D:D + 1, :])
                    bsb = nrm_sb.tile([64, QT_N], f32r, tag="bsb")
                    if use_pbc:
                        nc.gpsimd.partition_broadcast(bsb[:], rec[:])
                    else:
                        bps = mm_ps.tile([64, QT_N], f32, tag="mm", name="mmps")
                        nc.tensor.matmul(bps[:], ones_sb[:], rec[:],
                                         start=True, stop=True)
                        nc.vector.tensor_copy(bsb[:], bps[:])
                    nc.vector.tensor_mul(
                        outt[p][64 * hh:64 * hh + 64, QT_N * qt:QT_N * (qt + 1)],
                        ap[0:D, :], bsb[:])

            if phases == "a":
                for st in range(N_KT):
                    emit_v(st)
                for p in range(NP):
                    for sc in range(N_QT):
                        emit_qk_half(p, 0, sc)
                        emit_qk_half(p, 1, sc)
                for p in range(NP):
                    nc.sync.dma_start(
                        Y[(2 * p) * 128:(2 * p + 1) * 128, :],
                        qt_sb[p][:, 0:E])
                    nc.sync.dma_start(
                        Y[(2 * p + 1) * 128:(2 * p + 2) * 128, :],
                        kt_sb[p][:, 0:E])
                continue

            # qt-MAJOR schedule: all 4 head pairs at q-range qt, then the
            # output projection of qt. proj work spreads evenly across the
            # whole attention phase (the qt3 row alone has 32 kt2 groups to
            # hide proj(2) in) and the post-last-exp tail is just proj(3).
            # deadline key = sequential attention-slot index 4*qt + p.
            for p in range(NP):
                for sc in range(N_QT):
                    key = 4 * sc + p
                    push(key, lambda p=p, sc=sc: emit_qk_half(p, 1, sc))
                    push(key, lambda p=p, sc=sc: emit_qk_half(p, 0, sc))
            for st in range(N_KT):
                push(4 * (st // 4), lambda st=st: emit_v(st))
            for qt in range(N_QT):
                if qt == 1:
                    # Wout is consumed only by the qt3-row proj fillers;
                    # issuing its 2MB after the qt0 row keeps early DMA
                    # bandwidth for x/Wqkv, which gate the qt0/qt1 rows.
                    for p in range(NP):
                        nc.sync.dma_start(wout_sb[p][:],
                                          WOUT[128 * p:128 * (p + 1), :])
                for p in range(NP):
                    flush(4 * qt + p)
                    emit_attn(p, qt)
                if phases == "abc":
                    for st in range(4 * qt, 4 * (qt + 1)):
                        for et in range(E // 512):
                            push(16, lambda st=st, et=et:
                                 emit_proj_tile(st, et))
            flush(99)

            if phases == "ab":
                for p in range(NP):
                    nc.sync.dma_start(
                        Y[(2 * p) * 128:(2 * p + 1) * 128, :],
                        outt[p][:, 0:E])

    nc.compile()
    return nc


def _get_nc(n_et, repeat=1, phases="abc"):
    key = (n_et, repeat, phases)
    if key not in _CACHE:
        _CACHE[key] = _build(n_et, repeat, phases)
    return _CACHE[key]


def _shard(x, mask, Wqkv, bqkv, Wout, bout):
    """Host-side sharding: per-core input dicts."""
    import ml_dtypes

    bf16 = ml_dtypes.bfloat16
    x = np.asarray(x, dtype=np.float32)
    mask = np.asarray(mask)
    Wqkv = np.asarray(Wqkv, dtype=np.float32)
    bqkv = np.asarray(bqkv, dtype=np.float32)
    Wout = np.asarray(Wout, dtype=np.float32)

    has_bias = bool(np.any(bqkv))
    n_et = 9 if has_bias else 8
    E_pad = n_et * 128

    # diagonal wedge mask tiles (multiplicative 0/1), 128 columns each:
    # dmask[i, d, j] corresponds to mask[128*d + j, 128*d + i] -- the
    # partially-masked 128-wide wedge of diagonal block d within a q-tile.
    dmask = np.stack(
        [np.asarray(mask[128 * d:128 * (d + 1), 128 * d:128 * (d + 1)].T)
         for d in range(4)], axis=1).astype(np.float32)
    dmask = np.ascontiguousarray(dmask.astype(bf16))  # [128, 4, 128]

    in_maps = []
    for c in range(N_CORES):
        b, g = divmod(c, 2)
        heads = range(HG * g, HG * (g + 1))
        # per-group weight slices, feature order [Q heads | K heads | V heads]
        cols = []
        for blk in range(3):  # q, k, v blocks of Wqkv
            for h in heads:
                cols.append(Wqkv[:, blk * E + D * h: blk * E + D * h + D])
        wqkv_c = np.concatenate(cols, axis=1)  # [E, 3F]
        if has_bias:
            bias_cols = []
            for blk in range(3):
                for h in heads:
                    bias_cols.append(bqkv[blk * E + D * h: blk * E + D * h + D])
            brow = np.concatenate(bias_cols)[None, :]  # [1, 3F]
            wqkv_c = np.concatenate(
                [wqkv_c, brow, np.zeros((E_pad - E - 1, 3 * F), np.float32)], axis=0)
        xt_c = np.ascontiguousarray(x[b].T)  # [E, S]
        if has_bias:
            aug = np.zeros((E_pad - E, S), np.float32)
            aug[0, :] = 1.0
            xt_c = np.concatenate([xt_c, aug], axis=0)
        wout_c = np.ascontiguousarray(Wout[F * g:F * (g + 1), :])  # [F, E]
        in_maps.append({
            "xt": np.ascontiguousarray(xt_c.astype(bf16)),
            "wqkv": np.ascontiguousarray(wqkv_c.astype(bf16)),
            "wout": np.ascontiguousarray(wout_c.astype(bf16)),
            "dmask": dmask,
            "ones64": np.ones((1, 64), np.float32),
        })
    return in_maps, n_et


def run_sharded(inputs, trace=False):
    """Run the SPMD kernel; returns (y_full [B,S,E] f32, BassKernelResults)."""
    from concourse.bass_utils import run_bass_kernel_spmd

    in_maps, n_et = _shard(**inputs)
    nc = _get_nc(n_et)
    res = run_bass_kernel_spmd(nc, in_maps, core_ids=list(range(N_CORES)),
                               trace=trace)
    bout = np.asarray(inputs["bout"], dtype=np.float32)
    y = np.empty((B, S, E), np.float32)
    for b in range(B):
        y[b] = (res.results[2 * b]["y"].astype(np.float32)
                + res.results[2 * b + 1]["y"].astype(np.float32) + bout)
    return y, res


def kernel(**inputs) -> np.ndarray:
    y, _ = run_sharded(inputs, trace=False)
    return y
